# revision 13
# baseline (speedup 1.0000x reference)
"""Persistent-homology loss (coefficient of variation of the pairwise
distance matrix) on 8 TRN2 NeuronCores.

Math:
  X = embeddings.reshape(8192, 128)
  d2_ij = sq_i + sq_j - 2 X_i.X_j   (min off-diag d2 ~ 89, no clamp), d = sqrt(d2)
  out = std(d, ddof=1) / (mean(d) + 1e-8) over all N^2 entries.

Split:
  S2 = sum(d2) exactly on host in f64 via 2N*sum(sq) - 2*||sum(X)||^2.
  Host f64 also sums d over the 64 diagonal 128x128 chunk blocks and the
  64 ordered delta-32 blocks (the wrap column of the circulant window).
  Device computes the rest: for each own chunk rc, cols [r0+128, r0+4096)
  at weight 2 (every unordered off-diagonal pair exactly once).

Device strategy (SPMD, one program, per-core rolled data):
  - 8192 points in 64 chunks of 128; core c owns 8 chunks (1024 rows),
    inputs rolled by -1024c so offsets are program-uniform.
  - PE: single-shot bf16 matmuls u = X_i.X_j (start+stop, no PSUM
    read-modify-write -> ~0.74ns/col vs 1.4 for accumulate pairs).
  - Vector: one scalar_tensor_tensor per psum block:
    (u - 0.5 sq_i) - 0.5 sq_j into an SBUF tmp tile (~1.15ns/col).
    gpsimd cannot access PSUM, so vector takes all of them.
  - ACT: sqrt(tmp * -2) with accum_out per-partition row sums into
    partials [128, 16].
  - DMA: xt and sq each split into two partition-half DMAs on separate
    queues (single-queue DMA is ~130GB/s and otherwise gates the start).
  - Sync: walrus allows ONE semaphore wait per compute instruction.
    Tiny per-engine warmup reads (2 matmuls for the xt halves, 2 vector
    reads for the sq halves) establish DMA watermarks; same-engine waits
    are stripped post-build (in-order queues imply them); the closing
    Drain keeps only the out-DMA queue sem which transitively dominates
    everything (out <- ACT <- vector <- PE <- DMAs via warmups).
"""

import numpy as np
import ml_dtypes

import concourse.bass as bass
import concourse.tile as tile
from concourse import mybir
from concourse.bass_utils import run_bass_kernel_spmd

BF16 = ml_dtypes.bfloat16
N = 8192
D = 128
NCORES = 8
LOCAL = N // NCORES          # 1024 rows per core
NCHUNK = LOCAL // 128        # 8 row-chunks per core
XT_COLS = 4992               # max col touched: 7*128 + 128 + 3968
SQ_COLS = XT_COLS + NCHUNK   # + per-partition 0.5*sq_i column per chunk

F32 = mybir.dt.float32
MBF16 = mybir.dt.bfloat16
SQRT = mybir.ActivationFunctionType.Sqrt
ALU = mybir.AluOpType


def _build_nc() -> bass.Bass:
    nc = bass.Bass()
    xt_d = nc.declare_dram_parameter("xt", [D, XT_COLS], MBF16, isOutput=False)
    sq_d = nc.declare_dram_parameter("sq", [D, SQ_COLS], F32, isOutput=False)
    out_d = nc.declare_dram_parameter("out", [D, 16], F32, isOutput=True)

    with tile.TileContext(nc) as tc:
        with (
            tc.tile_pool(name="sb", bufs=1) as sb,
            tc.tile_pool(name="ob", bufs=2) as obp,
            tc.tile_pool(name="pp", bufs=2, space="PSUM") as pp,
        ):
            xt = sb.tile([D, XT_COLS], MBF16)
            sq = sb.tile([D, SQ_COLS], F32)
            partials = sb.tile([D, 16], F32)
            wv = sb.tile([1, 2], F32, name="wv")
            nc.sync.dma_start(xt[0:64, :], xt_d[0:64, :])
            nc.sync.dma_start(xt[64:128, :], xt_d[64:128, :])
            nc.sync.dma_start(sq[0:64, :], sq_d[0:64, :])
            nc.sync.dma_start(sq[64:128, :], sq_d[64:128, :])

            # warmups: establish each input-DMA watermark on its consumer
            # engine so every later instruction needs at most one wait.
            wps = pp.tile([D, 2048], F32, name="wps", tag="ps")
            nc.tensor.matmul(wps[0:1, 0:4], xt[0:64, 0:1], xt[0:64, 0:4],
                             start=True, stop=True)
            nc.tensor.matmul(wps[0:1, 4:8], xt[64:128, 0:1], xt[64:128, 0:4],
                             start=True, stop=True)
            nc.vector.tensor_scalar_add(wv[0:1, 0:1], sq[0:1, 0:1], 0.0)
            nc.vector.tensor_scalar_add(wv[0:1, 1:2], sq[64:65, 0:1], 0.0)

            tmps = [sb.tile([D, 3968], F32, name=f"tmp{rc}")
                    for rc in range(NCHUNK)]
            for rc in range(NCHUNK):
                r0 = rc * 128
                for h in range(2):
                    base = r0 + 128 + 2048 * h
                    w = 2048 if h == 0 else 1920
                    ps = pp.tile([D, 2048], F32, name=f"ps_{rc}_{h}", tag="ps")
                    off = 0
                    for seg in (512, 512, 512, 512 if h == 0 else 384):
                        nc.tensor.matmul(
                            ps[:, off:off + seg],
                            xt[:, r0:r0 + 128],
                            xt[:, base + off:base + off + seg],
                            start=True, stop=True,
                        )
                        off += seg
                    nc.vector.scalar_tensor_tensor(
                        tmps[rc][:, 2048 * h:2048 * h + w], ps[:, 0:w],
                        sq[:, XT_COLS + rc:XT_COLS + rc + 1],
                        sq[:, base:base + w],
                        ALU.subtract, ALU.subtract,
                    )
                o = obp.tile([D, 3968], MBF16, name=f"o{rc}", tag="o")
                nc.scalar.activation(
                    o[:, 0:2048], tmps[rc][:, 0:2048], SQRT, scale=-2.0,
                    accum_out=partials[:, 2 * rc:2 * rc + 1],
                )
                nc.scalar.activation(
                    o[:, 2048:3968], tmps[rc][:, 2048:3968], SQRT, scale=-2.0,
                    accum_out=partials[:, 2 * rc + 1:2 * rc + 2],
                )

            nc.sync.dma_start(out_d[:], partials[:])

    same = {"Activation": "Activation", "Matmult": "PE", "Ldweights": "PE",
            "TensorScalarPtr": "Pool", "ScalarTensorTensor": "Pool"}
    for inst in nc.all_instructions():
        si = inst.sync_info
        if not si or not si.on_wait:
            continue
        if inst.opcode == "Drain" and len(si.on_wait) > 1:
            keep = [w for w in si.on_wait if w.ant_name.startswith("DMAHW")]
            si.on_wait[:] = keep[-1:]
            continue
        pref = same.get(inst.opcode)
        if pref and len(si.on_wait) > 1:
            keep = [w for w in si.on_wait if not w.ant_name.startswith(pref)]
            if keep:
                si.on_wait[:] = keep
    return nc


def _host_prep(embeddings: np.ndarray):
    x = np.ascontiguousarray(embeddings.reshape(N, D).astype(np.float32))
    xT = np.ascontiguousarray(x.T)                      # [128, 8192] f32
    x64 = x.astype(np.float64)
    sq64 = np.einsum("ij,ij->i", x64, x64)              # [8192]
    ssum = x64.sum(axis=0)                              # [128]
    S2 = 2.0 * N * sq64.sum() - 2.0 * float(ssum @ ssum)

    # host f64: diagonal blocks (weight 1) + delta-32 blocks (each
    # unordered pair twice = both ordered blocks, via symmetry).
    S1_host = 0.0
    for g in range(N // 128):
        Xg = x64[128 * g:128 * g + 128]
        sg = sq64[128 * g:128 * g + 128]
        d2 = sg[:, None] + sg[None, :] - 2.0 * (Xg @ Xg.T)
        np.maximum(d2, 0.0, out=d2)
        S1_host += float(np.sqrt(d2).sum())
    for g in range(N // 256):
        a = slice(128 * g, 128 * g + 128)
        b = slice(128 * (g + 32), 128 * (g + 32) + 128)
        d2 = (sq64[a][:, None] + sq64[b][None, :]
              - 2.0 * (x64[a] @ x64[b].T))
        np.maximum(d2, 0.0, out=d2)
        S1_host += 2.0 * float(np.sqrt(d2).sum())

    half_sq = (0.5 * sq64).astype(np.float32)           # [8192]

    in_maps = []
    for c in range(NCORES):
        sh = -LOCAL * c
        xt_c = np.ascontiguousarray(
            np.roll(xT, sh, axis=1)[:, :XT_COLS].astype(BF16))
        hs = np.roll(half_sq, sh)
        sqv = np.empty((D, SQ_COLS), np.float32)
        sqv[:, :XT_COLS] = hs[None, :XT_COLS]
        for rc in range(NCHUNK):
            sqv[:, XT_COLS + rc] = hs[128 * rc:128 * rc + 128]
        in_maps.append({"xt": xt_c, "sq": sqv})
    return in_maps, S2, S1_host


def _combine(parts: list[np.ndarray], S2: float, S1_host: float) -> np.ndarray:
    S1 = S1_host
    for p in parts:
        S1 += 2.0 * p.astype(np.float64).sum()
    NN = float(N) * float(N)
    mean = S1 / NN
    var = (S2 - NN * mean * mean) / (NN - 1.0)
    return np.float32(np.sqrt(max(var, 0.0)) / (mean + 1e-8))


_NC_CACHE = None


def kernel(embeddings: np.ndarray) -> np.ndarray:
    global _NC_CACHE
    in_maps, S2, S1_host = _host_prep(embeddings)
    if _NC_CACHE is None:
        _NC_CACHE = _build_nc()
    res = run_bass_kernel_spmd(_NC_CACHE, in_maps, list(range(NCORES)))
    return _combine([r["out"] for r in res.results], S2, S1_host)


# revision 14
# speedup vs baseline: 1.1410x; 1.1410x over previous
"""Persistent-homology loss (coefficient of variation of the pairwise
distance matrix) on 8 TRN2 NeuronCores.

Math:
  X = embeddings.reshape(8192, 128)
  d2_ij = sq_i + sq_j - 2 X_i.X_j   (min off-diag d2 ~ 89, no clamp), d = sqrt(d2)
  out = std(d, ddof=1) / (mean(d) + 1e-8) over all N^2 entries.

Split:
  S2 = sum(d2) exactly on host in f64 via 2N*sum(sq) - 2*||sum(X)||^2.
  Host f64 also sums d over the 64 diagonal 128x128 chunk blocks and the
  64 ordered delta-32 blocks (the wrap column of the circulant window).
  Device computes the rest: for each own chunk rc, cols [r0+128, r0+4096)
  at weight 2 (every unordered off-diagonal pair exactly once).

Device strategy (SPMD, one program, per-core rolled data):
  - 8192 points in 64 chunks of 128; core c owns 8 chunks (1024 rows),
    inputs rolled by -1024c so offsets are program-uniform.
  - PE: single-shot bf16 matmuls u = X_i.X_j (start+stop, no PSUM
    read-modify-write -> ~0.74ns/col vs 1.4 for accumulate pairs).
  - Vector: one scalar_tensor_tensor per psum block:
    (u - 0.5 sq_i) - 0.5 sq_j into an SBUF tmp tile (~1.15ns/col).
    gpsimd cannot access PSUM, so vector takes all of them.
  - ACT: sqrt(tmp * -2) with accum_out per-partition row sums into
    partials [128, 16].
  - DMA: xt and sq each split into two partition-half DMAs on separate
    queues (single-queue DMA is ~130GB/s and otherwise gates the start).
  - Sync: walrus allows ONE semaphore wait per compute instruction.
    Tiny per-engine warmup reads (2 matmuls for the xt halves, 2 vector
    reads for the sq halves) establish DMA watermarks; same-engine waits
    are stripped post-build (in-order queues imply them); the closing
    Drain keeps only the out-DMA queue sem which transitively dominates
    everything (out <- ACT <- vector <- PE <- DMAs via warmups).
"""

import numpy as np
import ml_dtypes

import concourse.bass as bass
import concourse.tile as tile
from concourse import mybir
from concourse.bass_utils import run_bass_kernel_spmd

BF16 = ml_dtypes.bfloat16
N = 8192
D = 128
NCORES = 8
LOCAL = N // NCORES          # 1024 rows per core
NCHUNK = LOCAL // 128        # 8 row-chunks per core
XT_COLS = 4992               # max col touched: 7*128 + 128 + 3968
SQ_COLS = XT_COLS + NCHUNK   # + per-partition 0.5*sq_i column per chunk

F32 = mybir.dt.float32
MBF16 = mybir.dt.bfloat16
SQRT = mybir.ActivationFunctionType.Sqrt
ALU = mybir.AluOpType


def _build_nc() -> bass.Bass:
    nc = bass.Bass()
    xt_d = nc.declare_dram_parameter("xt", [D, XT_COLS], MBF16, isOutput=False)
    sq_d = nc.declare_dram_parameter("sq", [D, SQ_COLS], F32, isOutput=False)
    out_d = nc.declare_dram_parameter("out", [D, 16], F32, isOutput=True)

    with tile.TileContext(nc) as tc:
        with (
            tc.tile_pool(name="sb", bufs=1) as sb,
            tc.tile_pool(name="ob", bufs=2) as obp,
            tc.tile_pool(name="pp", bufs=2, space="PSUM") as pp,
        ):
            xt = sb.tile([D, XT_COLS], MBF16)
            sq = sb.tile([D, SQ_COLS], F32)
            partials = sb.tile([D, 16], F32)
            wv = sb.tile([1, 1], F32, name="wv")
            # DMA bandwidth is aggregate across queues (~190GB/s), so
            # splitting tensors across more queues does not help; two
            # whole-tensor DMAs measured fastest.
            nc.sync.dma_start(xt[:], xt_d[:])
            nc.sync.dma_start(sq[:], sq_d[:])
            # establish the sq-DMA watermark on the vector engine
            nc.vector.tensor_scalar_add(wv[0:1, 0:1], sq[0:1, 0:1], 0.0)

            tmps = [sb.tile([D, 3968], F32, name=f"tmp{rc}")
                    for rc in range(NCHUNK)]
            for rc in range(NCHUNK):
                r0 = rc * 128
                for h in range(2):
                    base = r0 + 128 + 2048 * h
                    w = 2048 if h == 0 else 1920
                    ps = pp.tile([D, 2048], F32, name=f"ps_{rc}_{h}", tag="ps")
                    off = 0
                    for seg in (512, 512, 512, 512 if h == 0 else 384):
                        nc.tensor.matmul(
                            ps[:, off:off + seg],
                            xt[:, r0:r0 + 128],
                            xt[:, base + off:base + off + seg],
                            start=True, stop=True,
                        )
                        off += seg
                    nc.vector.scalar_tensor_tensor(
                        tmps[rc][:, 2048 * h:2048 * h + w], ps[:, 0:w],
                        sq[:, XT_COLS + rc:XT_COLS + rc + 1],
                        sq[:, base:base + w],
                        ALU.subtract, ALU.subtract,
                    )
                o = obp.tile([D, 3968], MBF16, name=f"o{rc}", tag="o")
                nc.scalar.activation(
                    o[:, 0:2048], tmps[rc][:, 0:2048], SQRT, scale=-2.0,
                    accum_out=partials[:, 2 * rc:2 * rc + 1],
                )
                nc.scalar.activation(
                    o[:, 2048:3968], tmps[rc][:, 2048:3968], SQRT, scale=-2.0,
                    accum_out=partials[:, 2 * rc + 1:2 * rc + 2],
                )

            nc.sync.dma_start(out_d[:], partials[:])

    same = {"Activation": "Activation", "Matmult": "PE", "Ldweights": "PE",
            "TensorScalarPtr": "Pool", "ScalarTensorTensor": "Pool"}
    for inst in nc.all_instructions():
        si = inst.sync_info
        if not si or not si.on_wait:
            continue
        if inst.opcode == "Drain" and len(si.on_wait) > 1:
            keep = [w for w in si.on_wait if w.ant_name.startswith("DMAHW")]
            si.on_wait[:] = keep[-1:]
            continue
        pref = same.get(inst.opcode)
        if pref and len(si.on_wait) > 1:
            keep = [w for w in si.on_wait if not w.ant_name.startswith(pref)]
            if keep:
                si.on_wait[:] = keep
    return nc


def _host_prep(embeddings: np.ndarray):
    x = np.ascontiguousarray(embeddings.reshape(N, D).astype(np.float32))
    xT = np.ascontiguousarray(x.T)                      # [128, 8192] f32
    x64 = x.astype(np.float64)
    sq64 = np.einsum("ij,ij->i", x64, x64)              # [8192]
    ssum = x64.sum(axis=0)                              # [128]
    S2 = 2.0 * N * sq64.sum() - 2.0 * float(ssum @ ssum)

    # host f64: diagonal blocks (weight 1) + delta-32 blocks (each
    # unordered pair twice = both ordered blocks, via symmetry).
    S1_host = 0.0
    for g in range(N // 128):
        Xg = x64[128 * g:128 * g + 128]
        sg = sq64[128 * g:128 * g + 128]
        d2 = sg[:, None] + sg[None, :] - 2.0 * (Xg @ Xg.T)
        np.maximum(d2, 0.0, out=d2)
        S1_host += float(np.sqrt(d2).sum())
    for g in range(N // 256):
        a = slice(128 * g, 128 * g + 128)
        b = slice(128 * (g + 32), 128 * (g + 32) + 128)
        d2 = (sq64[a][:, None] + sq64[b][None, :]
              - 2.0 * (x64[a] @ x64[b].T))
        np.maximum(d2, 0.0, out=d2)
        S1_host += 2.0 * float(np.sqrt(d2).sum())

    half_sq = (0.5 * sq64).astype(np.float32)           # [8192]

    in_maps = []
    for c in range(NCORES):
        sh = -LOCAL * c
        xt_c = np.ascontiguousarray(
            np.roll(xT, sh, axis=1)[:, :XT_COLS].astype(BF16))
        hs = np.roll(half_sq, sh)
        sqv = np.empty((D, SQ_COLS), np.float32)
        sqv[:, :XT_COLS] = hs[None, :XT_COLS]
        for rc in range(NCHUNK):
            sqv[:, XT_COLS + rc] = hs[128 * rc:128 * rc + 128]
        in_maps.append({"xt": xt_c, "sq": sqv})
    return in_maps, S2, S1_host


def _combine(parts: list[np.ndarray], S2: float, S1_host: float) -> np.ndarray:
    S1 = S1_host
    for p in parts:
        S1 += 2.0 * p.astype(np.float64).sum()
    NN = float(N) * float(N)
    mean = S1 / NN
    var = (S2 - NN * mean * mean) / (NN - 1.0)
    return np.float32(np.sqrt(max(var, 0.0)) / (mean + 1e-8))


_NC_CACHE = None


def kernel(embeddings: np.ndarray) -> np.ndarray:
    global _NC_CACHE
    in_maps, S2, S1_host = _host_prep(embeddings)
    if _NC_CACHE is None:
        _NC_CACHE = _build_nc()
    res = run_bass_kernel_spmd(_NC_CACHE, in_maps, list(range(NCORES)))
    return _combine([r["out"] for r in res.results], S2, S1_host)
